# revision 27
# baseline (speedup 1.0000x reference)
"""Trainium2 Bass kernel for nn_BlockCross (encoder-decoder transformer block).

Strategy: pure data parallel over batch (B=256 -> 32 per core x 8 cores).
Residual stream kept feature-major [C(partitions), tokens(free)] so every
linear is matmul(lhsT=W[Cin,Cout], rhs=x_fm) with zero transposes.
Attention uses transposed scores s_T[S, Tq] = k_fm^T-free matmul, exp on ACT
(with the 1/sqrt(C) scale folded in), softmax sums fused into the attn@v
matmul as an extra all-ones column of v (token-major v), per-query-token
normalization via per-partition tensor_scalar on the token-major output, then
a PE transpose back to feature-major for the output projection.
Matmuls run in float32r (full fp32 storage, ~1.6e-4 matmul error, 4x faster
than fp32); small decoder-side attention cores and the FFN W2/h path use bf16.
LayerNorm gammas are folded into the following weight matrices on the host
(all betas/biases in this problem are zero; asserted).
"""

import sys

sys.path.insert(0, "/opt/trn_rl_repo")

import numpy as np
import ml_dtypes

C = 384
H = 6
HS = 64
B = 256
T = 64
S = 256
EPS = 1e-5
SCALE = float(C) ** -0.5
NCORES = 8
BL = B // NCORES          # 32 batch elems per core
ENC_TOK = BL * S          # 8192
DEC_TOK = BL * T          # 2048
ENC_PAIRS = ENC_TOK // 512   # 16 iterations, 2 elems each
DEC_TILES = DEC_TOK // 512   # 4 iterations, 8 elems each

_CACHE = {}


def _split_multi_waits(nc, mybir, bass_rust):
    """This toolchain's walrus accepts at most ONE sync wait per instruction;
    Tile's tail drain (and some others) aggregate several. Hoist extras onto
    single-wait NoOps inserted just before, on the same engine."""
    n_fixed = 0
    for fn in nc.m.functions:
        for bb in fn.blocks:
            insts = bb.instructions
            new_list = []
            changed = False
            for inst in insts:
                si = inst.sync_info
                waits = list(si.on_wait) if si is not None else []
                if len(waits) > 1:
                    for w in waits[:-1]:
                        nop = mybir.InstNoOp(name=f"I-waitsplit-{nc.next_id()}",
                                             ins=[], outs=[], engine=inst.engine)
                        nop.sync_info = bass_rust.SyncInfo(on_wait=[w], on_update=[])
                        new_list.append(nop)
                        nc.register_instruction(nop)
                        n_fixed += 1
                    si.on_wait = [waits[-1]]
                    inst.sync_info = si
                    changed = True
                new_list.append(inst)
            if changed:
                bb.instructions = new_list
    return n_fixed


def _build_program():
    import concourse.bass as bass
    import concourse.mybir as mybir
    import bass_rust
    from concourse.tile import TileContext
    from concourse.masks import make_identity

    F32R = mybir.dt.float32r
    BF16 = mybir.dt.bfloat16
    FP32 = mybir.dt.float32
    AF = mybir.ActivationFunctionType

    nc = bass.Bass()

    # ---------- DRAM tensors ----------
    x_in_d = nc.dram_tensor("x_in_fm", [3, 128, ENC_TOK], F32R, kind="ExternalInput")
    x_out_d = nc.dram_tensor("x_out_fm", [3, 128, DEC_TOK], F32R, kind="ExternalInput")
    mask_d = nc.dram_tensor("mask", [128, 128], BF16, kind="ExternalInput")

    def wdram(name, kc, n, dt):
        return nc.dram_tensor(name, [kc, 128, n], dt, kind="ExternalInput")

    wq_mc_d = wdram("wq_mc", 3, 384, F32R)
    wk_mc_d = wdram("wk_mc", 3, 384, F32R)
    wv_mc_d = wdram("wv_mc", 3, 384, F32R)
    wp_mc_d = wdram("wp_mc", 3, 384, F32R)
    w1_fc_d = wdram("w1_fc", 3, 1536, F32R)
    w2_fc_d = wdram("w2_fc", 12, 384, BF16)
    wq_mm_d = wdram("wq_mm", 3, 384, F32R)
    wk_mm_d = wdram("wk_mm", 3, 384, F32R)
    wv_mm_d = wdram("wv_mm", 3, 384, F32R)
    wp_mm_d = wdram("wp_mm", 3, 384, F32R)
    wq_m_d = wdram("wq_m", 3, 384, F32R)
    wk_m_d = wdram("wk_m", 3, 384, F32R)
    wv_m_d = wdram("wv_m", 3, 384, F32R)
    wp_m_d = wdram("wp_m", 3, 384, F32R)
    w1_ff_d = wdram("w1_ff", 3, 1536, F32R)
    w2_ff_d = wdram("w2_ff", 12, 384, BF16)

    cinvC_d = nc.dram_tensor("cinvC", [128, 1], F32R, kind="ExternalInput")
    cones_d = nc.dram_tensor("cones", [1, 128], F32R, kind="ExternalInput")

    xin_o = nc.dram_tensor("xin_o", [3, 128, ENC_TOK], FP32, kind="ExternalOutput")
    xo_o = nc.dram_tensor("xo_o", [3, 128, DEC_TOK], FP32, kind="ExternalOutput")

    # scratch for cross-attention K/V computed from final encoder output
    kx_d = nc.dram_tensor("kx_d", [3, 128, ENC_TOK], BF16)
    vx_d = nc.dram_tensor("vx_d", [ENC_TOK // 128, 128, 6 * 66], BF16)

    with TileContext(nc) as tc:
        with tc.tile_pool(name="cons", bufs=1) as cons, \
             tc.tile_pool(name="pa", bufs=2) as pa, \
             tc.tile_pool(name="pz", bufs=1) as pz, \
             tc.tile_pool(name="pr3", bufs=3) as pr3, \
             tc.tile_pool(name="ph", bufs=1) as ph, \
             tc.tile_pool(name="pb", bufs=2) as pb, \
             tc.tile_pool(name="pr", bufs=1) as pr, \
             tc.tile_pool(name="psA", bufs=3, space="PSUM") as psA, \
             tc.tile_pool(name="psV", bufs=2, space="PSUM") as psV, \
             tc.tile_pool(name="psT", bufs=2, space="PSUM") as psT:

            invC = cons.tile([128, 1], F32R)
            nc.sync.dma_start(out=invC, in_=cinvC_d[:, :])
            ones1 = cons.tile([1, 128], F32R)
            nc.sync.dma_start(out=ones1, in_=cones_d[:, :])
            epsb = cons.tile([128, 1], FP32)
            nc.vector.memset(epsb, EPS)
            ident = cons.tile([128, 128], BF16)
            make_identity(nc, ident)
            # block-diagonal causal mask for pair-fused decoder self-attention:
            # mask[s, t] = (s//64 == t//64) and (s%64 <= t%64)
            mask_sb = cons.tile([128, 128], BF16)
            nc.sync.dma_start(out=mask_sb, in_=mask_d[:, :])

            # ---------- helpers ----------
            def load_w(pool, dram, kc, n, dt, tag):
                t = pool.tile([128, kc, n], dt, tag=tag)
                for c in range(kc):
                    nc.sync.dma_start(out=t[:, c, :], in_=dram[c, :, :])
                return t

            def ln_block(x, TW, ztag):
                """x: [128,3,TW] f32r feature-major -> z = (x-m)*rstd, f32r.

                rows layout: 0=mean 1=E[x^2] 2=m^2 3=var 4=std 5=rstd 6=m*rstd
                """
                xf = x.rearrange("p c t -> p (c t)")
                sq = pz.tile([128, 3, TW], F32R, tag="scr")
                nc.vector.tensor_mul(sq.rearrange("p c t -> p (c t)"), xf, xf)
                statA = psA.tile([1, TW], FP32, tag="mm")
                for c in range(3):
                    nc.tensor.matmul(statA, invC, x[:, c, :],
                                     start=(c == 0), stop=(c == 2))
                statB = psA.tile([1, TW], FP32, tag="mm")
                for c in range(3):
                    nc.tensor.matmul(statB, invC, sq[:, c, :],
                                     start=(c == 0), stop=(c == 2))
                rowsT = pr.tile([1, 5, TW], FP32, tag="rowsT")
                nc.vector.tensor_copy(rowsT[:, 0, :], statA)
                nc.vector.tensor_copy(rowsT[:, 1, :], statB)
                nc.vector.tensor_mul(rowsT[:, 2, :], rowsT[:, 0, :], rowsT[:, 0, :])
                nc.vector.tensor_sub(rowsT[:, 3, :], rowsT[:, 1, :], rowsT[:, 2, :])
                nc.scalar.activation(rowsT[:, 4, :], rowsT[:, 3, :], AF.Sqrt,
                                     bias=epsb[0:1, :])
                rowsF = pr.tile([1, 2, TW], F32R, tag="rowsF")
                with nc.allow_low_precision(reason="f32r rstd feeds f32r matmul"):
                    nc.vector.reciprocal(rowsF[:, 0, :], rowsT[:, 4, :])
                nc.vector.tensor_mul(rowsF[:, 1, :], rowsT[:, 0, :].bitcast(F32R),
                                     rowsF[:, 0, :])
                bcR = psA.tile([128, TW], FP32, tag="mm")
                nc.tensor.matmul(bcR, ones1, rowsF[:, 0, :], start=True, stop=True)
                bcM = psA.tile([128, TW], FP32, tag="mm")
                nc.tensor.matmul(bcM, ones1, rowsF[:, 1, :], start=True, stop=True)
                tmp = pz.tile([128, 3, TW], F32R, tag="scr")
                nc.vector.tensor_mul(tmp, x,
                                     bcR.bitcast(F32R).unsqueeze(1).broadcast_to([128, 3, TW]))
                z = pz.tile([128, 3, TW], F32R, tag=ztag)
                nc.vector.tensor_sub(z, tmp,
                                     bcM.bitcast(F32R).unsqueeze(1).broadcast_to([128, 3, TW]))
                return z

            def linear_fm(z, w_sb, n_kc, n_out, TW, evac):
                """out[m] = sum_kc w[:,kc,m*128:...]^T @ z[:,kc,:]; evac(m, psum)."""
                for m in range(n_out // 128):
                    pp = psA.tile([128, TW], FP32, tag="mm")
                    for c in range(n_kc):
                        nc.tensor.matmul(pp, w_sb[:, c, m * 128:(m + 1) * 128],
                                         z[:, c, :], start=(c == 0), stop=(c == n_kc - 1))
                    evac(m, pp)
                return

            def v_proj(z, w_sb, TW, vtag):
                """token-major v with fused all-ones column: [128, TW/128, 6, 66]."""
                ntc = TW // 128
                v_sb = pa.tile([128, ntc, 6, 66], BF16, tag=vtag)
                for tcc in range(ntc):
                    pv = psV.tile([128, 384], FP32, tag="v")
                    for c in range(3):
                        nc.tensor.matmul(pv, z[:, c, tcc * 128:(tcc + 1) * 128],
                                         w_sb[:, c, :], start=(c == 0), stop=(c == 2))
                    nc.vector.tensor_copy(v_sb[:, tcc, :, 0:64],
                                          pv.rearrange("p (a b) -> p a b", a=6))
                    nc.vector.memset(v_sb[:, tcc, :, 64:65], 1.0)
                return v_sb

            def evac_to(dst, dt_cast=None):
                def f(m, pp):
                    nc.scalar.activation(dst[:, m, :], pp, AF.Copy)
                return f

            def evac_relu(dst):
                def f(m, pp):
                    nc.scalar.activation(dst[:, m, :], pp, AF.Relu)
                return f

            def evac_resid(dst, res):
                def f(m, pp):
                    nc.vector.tensor_add(dst[:, m, :], pp.bitcast(F32R), res[:, m, :])
                return f

            def attn_tail(o_ps, onm, base, nr):
                """normalize token-major attn output rows [base:base+nr].
                o_ps slice, rcp slice and onm slice all share the same
                partition base (engines cannot move data across partitions)."""
                rcp = pb.tile([128, 6], FP32, tag="rcp")
                ops = o_ps[base:base + nr] if o_ps.shape[0] == 128 else o_ps
                nc.vector.reciprocal(rcp[base:base + nr, :], ops[:, :, 64])
                nc.vector.tensor_mul(
                    onm[base:base + nr, :, :], ops[:, :, 0:64],
                    rcp[base:base + nr, :].unsqueeze(2).broadcast_to([nr, 6, 64]))

            def transpose_unit(onm, ofm_sb, col):
                """onm [128, 6, 64] bf16 token-major -> ofm_sb[:, :, col:col+128] f32r."""
                tp = psT.tile([128, 3, 128], BF16, tag="tp")
                onf = onm.rearrange("p a b -> p (a b)")
                for c in range(3):
                    nc.tensor.transpose(tp[:, c, :], onf[:, c * 128:(c + 1) * 128], ident)
                nc.vector.tensor_copy(ofm_sb[:, :, col:col + 128], tp)

            # =========================================================
            # Phase 1: encoder (mc self-attention + ffc FFN), per 2-elem pair
            # =========================================================
            with tc.tile_pool(name="pwE", bufs=1) as pw:
                wq_mc = load_w(pw, wq_mc_d, 3, 384, F32R, "wq")
                wk_mc = load_w(pw, wk_mc_d, 3, 384, F32R, "wk")
                wv_mc = load_w(pw, wv_mc_d, 3, 384, F32R, "wv")
                wp_mc = load_w(pw, wp_mc_d, 3, 384, F32R, "wp")
                w1_fc = load_w(pw, w1_fc_d, 3, 1536, F32R, "w1")
                w2_fc = load_w(pw, w2_fc_d, 12, 384, BF16, "w2")
                wk_m = load_w(pw, wk_m_d, 3, 384, F32R, "wkm")
                wv_m = load_w(pw, wv_m_d, 3, 384, F32R, "wvm")

                for p in range(ENC_PAIRS):
                    sl = slice(p * 512, (p + 1) * 512)
                    x = pa.tile([128, 3, 512], F32R, tag="x")
                    for c in range(3):
                        nc.sync.dma_start(out=x[:, c, :], in_=x_in_d[c, :, sl])
                    z1 = ln_block(x, 512, "z")
                    q_sb = pa.tile([128, 3, 512], F32R, tag="q")
                    linear_fm(z1, wq_mc, 3, 384, 512, evac_to(q_sb))
                    k_sb = pa.tile([128, 3, 512], F32R, tag="k")
                    linear_fm(z1, wk_mc, 3, 384, 512, evac_to(k_sb))
                    v_sb = v_proj(z1, wv_mc, 512, "v")

                    ofm = pa.tile([128, 3, 512], F32R, tag="ofm")
                    for e in range(2):
                        o_ps = [psV.tile([128, 6, 66], FP32, tag="v", name=f"ops{p}_{e}_{t2}")
                                for t2 in range(2)]
                        for h in range(6):
                            hb = (h % 2) * 64
                            hc = h // 2
                            sps = psA.tile([128, 2, 256], FP32, tag="mm")
                            for sc in range(2):
                                nc.tensor.matmul(
                                    sps[:, sc, :],
                                    k_sb[hb:hb + 64, hc, e * 256 + sc * 128: e * 256 + (sc + 1) * 128],
                                    q_sb[hb:hb + 64, hc, e * 256:(e + 1) * 256],
                                    start=True, stop=True)
                            e_sb = pb.tile([128, 2, 256], BF16, tag="e")
                            nc.scalar.activation(e_sb.rearrange("p a b -> p (a b)"),
                                                 sps.rearrange("p a b -> p (a b)"),
                                                 AF.Exp, scale=SCALE)
                            for tcc in range(2):
                                for sc in range(2):
                                    nc.tensor.matmul(
                                        o_ps[tcc][:, h, 0:65],
                                        e_sb[:, sc, tcc * 128:(tcc + 1) * 128],
                                        v_sb[:, 2 * e + sc, h, 0:65],
                                        start=(sc == 0), stop=(sc == 1))
                        for tcc in range(2):
                            onm = pb.tile([128, 6, 64], BF16, tag="onm")
                            attn_tail(o_ps[tcc], onm, 0, 128)
                            transpose_unit(onm, ofm, (2 * e + tcc) * 128)

                    xin1 = pr3.tile([128, 3, 512], F32R, tag="r")
                    linear_fm(ofm, wp_mc, 3, 384, 512, evac_resid(xin1, x))
                    z2 = ln_block(xin1, 512, "z")
                    h_sb = ph.tile([128, 12, 512], BF16, tag="h")
                    linear_fm(z2, w1_fc, 3, 1536, 512, evac_relu(h_sb))
                    xin2 = pr3.tile([128, 3, 512], F32R, tag="r")
                    linear_fm(h_sb, w2_fc, 12, 384, 512, evac_resid(xin2, xin1))
                    for c in range(3):
                        nc.sync.dma_start(out=xin_o[c, :, sl], in_=xin2[:, c, :].bitcast(FP32))

                    # cross-attention K/V from final encoder stream
                    kx_sb = pb.tile([128, 3, 512], BF16, tag="kx")
                    linear_fm(xin2, wk_m, 3, 384, 512, evac_to(kx_sb))
                    for c in range(3):
                        nc.sync.dma_start(out=kx_d[c, :, sl], in_=kx_sb[:, c, :])
                    vx_sb = v_proj(xin2, wv_m, 512, "v_sb2")
                    for tcc in range(4):
                        nc.sync.dma_start(
                            out=vx_d[p * 4 + tcc, :, :],
                            in_=vx_sb[:, tcc, :, :].rearrange("p a b -> p (a b)"))

            # =========================================================
            # Phase 2: decoder (mm masked self-attn, m cross-attn, ff FFN)
            # =========================================================
            with tc.tile_pool(name="pwD", bufs=1) as pw:
                wq_mm = load_w(pw, wq_mm_d, 3, 384, F32R, "wq")
                wk_mm = load_w(pw, wk_mm_d, 3, 384, F32R, "wk")
                wv_mm = load_w(pw, wv_mm_d, 3, 384, F32R, "wv")
                wp_mm = load_w(pw, wp_mm_d, 3, 384, F32R, "wp")
                wq_m = load_w(pw, wq_m_d, 3, 384, F32R, "wqm")
                wp_m = load_w(pw, wp_m_d, 3, 384, F32R, "wpm")
                w1_ff = load_w(pw, w1_ff_d, 3, 1536, F32R, "w1")
                w2_ff = load_w(pw, w2_ff_d, 12, 384, BF16, "w2")

                for d in range(DEC_TILES):
                    sl = slice(d * 512, (d + 1) * 512)
                    x = pa.tile([128, 3, 512], F32R, tag="x")
                    for c in range(3):
                        nc.sync.dma_start(out=x[:, c, :], in_=x_out_d[c, :, sl])
                    z1 = ln_block(x, 512, "z")
                    qd = pa.tile([128, 3, 512], BF16, tag="q")
                    linear_fm(z1, wq_mm, 3, 384, 512, evac_to(qd))
                    kd = pa.tile([128, 3, 512], BF16, tag="k")
                    linear_fm(z1, wk_mm, 3, 384, 512, evac_to(kd))
                    vd = v_proj(z1, wv_mm, 512, "v")

                    # masked self-attention, fused over elem PAIRS: the pair's
                    # 128 tokens form one block; the block-diagonal causal mask
                    # zeroes cross-elem terms after exp, so the fused attn@v
                    # (contracting all 128 keys) and its fused ones-column sums
                    # stay exact. Everything runs at partition base 0.
                    ofm1 = pa.tile([128, 3, 512], F32R, tag="ofm")
                    for prr in range(4):
                        onm = pb.tile([128, 6, 64], BF16, tag="onm")
                        o_ps = psV.tile([128, 6, 66], FP32, tag="v")
                        for h in range(6):
                            hb = (h % 2) * 64
                            hc = h // 2
                            sps = psA.tile([128, 128], FP32, tag="mm")
                            nc.tensor.matmul(sps,
                                             kd[hb:hb + 64, hc, prr * 128:(prr + 1) * 128],
                                             qd[hb:hb + 64, hc, prr * 128:(prr + 1) * 128],
                                             start=True, stop=True)
                            et = pb.tile([128, 128], BF16, tag="e")
                            nc.scalar.activation(et, sps, AF.Exp, scale=SCALE)
                            etm = pb.tile([128, 128], BF16, tag="e2")
                            nc.vector.tensor_mul(etm, et, mask_sb)
                            nc.tensor.matmul(o_ps[:, h, 0:65], etm,
                                             vd[:, prr, h, 0:65],
                                             start=True, stop=True)
                        attn_tail(o_ps, onm, 0, 128)
                        transpose_unit(onm, ofm1, prr * 128)

                    xo1 = pr3.tile([128, 3, 512], F32R, tag="r")
                    linear_fm(ofm1, wp_mm, 3, 384, 512, evac_resid(xo1, x))
                    z2 = ln_block(xo1, 512, "z")
                    qm = pa.tile([128, 3, 512], BF16, tag="k")
                    linear_fm(z2, wq_m, 3, 384, 512, evac_to(qm))

                    # cross-attention, per elem (keys/values streamed from scratch)
                    ofm2 = pa.tile([128, 3, 512], F32R, tag="ofm")
                    for e in range(8):
                        ge = d * 8 + e
                        if e % 2 == 0:
                            onm = pb.tile([128, 6, 64], BF16, tag="onm")
                        kxe = pb.tile([128, 3, 256], BF16, tag="kx")
                        for c in range(3):
                            nc.sync.dma_start(out=kxe[:, c, :],
                                              in_=kx_d[c, :, ge * 256:(ge + 1) * 256])
                        vxe = pb.tile([128, 2, 6, 66], BF16, tag="vxe")
                        for sc in range(2):
                            nc.sync.dma_start(
                                out=vxe[:, sc, :, :].rearrange("p a b -> p (a b)"),
                                in_=vx_d[ge * 2 + sc, :, :])
                        eb = (e % 2) * 64
                        o_ps = psV.tile([128, 6, 66], FP32, tag="v")
                        for h in range(6):
                            hb = (h % 2) * 64
                            hc = h // 2
                            sps = psA.tile([128, 2, 64], FP32, tag="mm")
                            for sc in range(2):
                                nc.tensor.matmul(sps[:, sc, :],
                                                 kxe[hb:hb + 64, hc, sc * 128:(sc + 1) * 128],
                                                 qm[hb:hb + 64, hc, e * 64:(e + 1) * 64],
                                                 start=True, stop=True)
                            ex = pb.tile([128, 2, 64], BF16, tag="e")
                            nc.scalar.activation(ex.rearrange("p a b -> p (a b)"),
                                                 sps.rearrange("p a b -> p (a b)"),
                                                 AF.Exp, scale=SCALE)
                            for sc in range(2):
                                nc.tensor.matmul(o_ps[eb:eb + 64, h, 0:65],
                                                 ex[:, sc, :],
                                                 vxe[:, sc, h, 0:65],
                                                 start=(sc == 0), stop=(sc == 1))
                        attn_tail(o_ps, onm, eb, 64)
                        if e % 2 == 1:
                            transpose_unit(onm, ofm2, (e // 2) * 128)

                    xo2 = pr3.tile([128, 3, 512], F32R, tag="r")
                    linear_fm(ofm2, wp_m, 3, 384, 512, evac_resid(xo2, xo1))
                    z3 = ln_block(xo2, 512, "z")
                    h_sb = ph.tile([128, 12, 512], BF16, tag="h")
                    linear_fm(z3, w1_ff, 3, 1536, 512, evac_relu(h_sb))
                    xo3 = pr3.tile([128, 3, 512], F32R, tag="r")
                    linear_fm(h_sb, w2_ff, 12, 384, 512, evac_resid(xo3, xo2))
                    for c in range(3):
                        nc.sync.dma_start(out=xo_o[c, :, sl], in_=xo3[:, c, :].bitcast(FP32))

    _split_multi_waits(nc, mybir, bass_rust)
    nc.finalize()
    return nc


def _prep_host(x_out, x_in, params):
    """Fold LN gammas into weights, check zero biases, build per-core arrays."""
    def npa(v):
        return np.asarray(v, dtype=np.float32)

    p = params
    for lnname in ["ln1", "ln2", "ln3", "ln1c", "ln2c"]:
        assert not np.any(npa(p[lnname]["b"])), f"{lnname}.b nonzero; unsupported"
    for mod in ["mm", "m", "mc"]:
        assert not np.any(npa(p[mod]["bp"])), f"{mod}.bp nonzero; unsupported"
    for mod in ["ff", "ffc"]:
        assert not np.any(npa(p[mod]["b1"])), f"{mod}.b1 nonzero; unsupported"
        assert not np.any(npa(p[mod]["b2"])), f"{mod}.b2 nonzero; unsupported"

    def wcat(w):  # [H, C, HS] -> [C, H*HS]
        return npa(w).transpose(1, 0, 2).reshape(C, C)

    def fold(wflat, g):  # row-scale by LN gamma
        return (npa(g)[:, None] * wflat).astype(np.float32)

    def chunk3(w):
        return np.ascontiguousarray(w.reshape(3, 128, -1))

    def chunk12(w, dt=np.float32):
        return np.ascontiguousarray(w.reshape(12, 128, -1).astype(dt))

    g1c = p["ln1c"]["g"]; g2c = p["ln2c"]["g"]
    g1 = p["ln1"]["g"]; g2 = p["ln2"]["g"]; g3 = p["ln3"]["g"]

    weights = {
        "wq_mc": chunk3(fold(wcat(p["mc"]["wq"]), g1c)),
        "wk_mc": chunk3(fold(wcat(p["mc"]["wk"]), g1c)),
        "wv_mc": chunk3(fold(wcat(p["mc"]["wv"]), g1c)),
        "wp_mc": chunk3(npa(p["mc"]["wp"])),
        "w1_fc": chunk3(fold(npa(p["ffc"]["w1"]), g2c)),
        "w2_fc": chunk12(npa(p["ffc"]["w2"]), ml_dtypes.bfloat16),
        "wq_mm": chunk3(fold(wcat(p["mm"]["wq"]), g1)),
        "wk_mm": chunk3(fold(wcat(p["mm"]["wk"]), g1)),
        "wv_mm": chunk3(fold(wcat(p["mm"]["wv"]), g1)),
        "wp_mm": chunk3(npa(p["mm"]["wp"])),
        "wq_m": chunk3(fold(wcat(p["m"]["wq"]), g2)),
        "wk_m": chunk3(wcat(p["m"]["wk"])),
        "wv_m": chunk3(wcat(p["m"]["wv"])),
        "wp_m": chunk3(npa(p["m"]["wp"])),
        "w1_ff": chunk3(fold(npa(p["ff"]["w1"]), g3)),
        "w2_ff": chunk12(npa(p["ff"]["w2"]), ml_dtypes.bfloat16),
    }

    # block-diagonal causal mask over an elem-pair's 128 tokens:
    # mask[s, t] = 1 iff same elem (s//64 == t//64) and s%64 <= t%64
    idx = np.arange(128)
    same = (idx[:, None] // 64) == (idx[None, :] // 64)
    causal = (idx[:, None] % 64) <= (idx[None, :] % 64)
    mask = np.ascontiguousarray((same & causal).astype(ml_dtypes.bfloat16))

    x_in = np.asarray(x_in, dtype=np.float32)
    x_out = np.asarray(x_out, dtype=np.float32)
    in_maps = []
    for ci in range(NCORES):
        xi = x_in[ci * BL:(ci + 1) * BL]          # [BL, S, C]
        xo = x_out[ci * BL:(ci + 1) * BL]         # [BL, T, C]
        xi_fm = np.ascontiguousarray(xi.transpose(2, 0, 1).reshape(3, 128, ENC_TOK))
        xo_fm = np.ascontiguousarray(xo.transpose(2, 0, 1).reshape(3, 128, DEC_TOK))
        m = {"x_in_fm": xi_fm, "x_out_fm": xo_fm, "mask": mask,
             "cinvC": np.full((128, 1), 1.0 / C, np.float32),
             "cones": np.ones((1, 128), np.float32)}
        m.update(weights)
        in_maps.append(m)
    return in_maps


def kernel(x_out, x_in, params):
    from concourse.bass_utils import run_bass_kernel_spmd

    if "nc" not in _CACHE:
        _CACHE["nc"] = _build_program()
    nc = _CACHE["nc"]

    in_maps = _prep_host(x_out, x_in, params)
    res = run_bass_kernel_spmd(nc, in_maps, list(range(NCORES)))

    xin_full = np.empty((B, S, C), np.float32)
    xo_full = np.empty((B, T, C), np.float32)
    for ci in range(NCORES):
        r = res.results[ci]
        xin_full[ci * BL:(ci + 1) * BL] = (
            r["xin_o"].reshape(C, BL, S).transpose(1, 2, 0))
        xo_full[ci * BL:(ci + 1) * BL] = (
            r["xo_o"].reshape(C, BL, T).transpose(1, 2, 0))
    return xo_full, xin_full
